# revision 2
# baseline (speedup 1.0000x reference)
"""Trainium2 Bass kernel for DictionaryLabelAttention.

Reference computation (B=4, S=4096, D=768, M=3072, C=8921):
    f_note = relu(x @ w_enc + b_enc)                       # (B,S,M)
    x_hat  = f_note @ w_dec + b_dec                        # (B,S,D)
    loss   = mean((x_hat-x)^2) + 1e-3*mean(|f_note|)
    attn   = softmax_S(f_note @ icd.T)                     # (B,S,C)
    logits = einsum('bsc,bsd,cd->bc', attn, x, out_w) + out_b

Sharding: 8 cores = 4 batches x 2 label-halves. Each core computes its
batch's f_note once (fp32r matmuls, full PE rate), streams its icd/out_w
label slice, and runs a streaming softmax over S (no max subtraction --
logits are O(3), exp is safe):
    den[c] = sum_s exp(l[c,s]);  num[c] = sum_s exp(l[c,s]) * (x[s].out_w[c])
    logits[c] = num[c]/den[c]
The (B,S,C) attention tensor never touches HBM. Loss partials are
reduced on-chip to 2 scalars/core and combined on host.
"""
import numpy as np

import concourse.bass as bass
import concourse.mybir as mybir
import concourse.tile as tile
from concourse import bacc
from concourse.bass_utils import run_bass_kernel_spmd

F32 = mybir.dt.float32
F32R = mybir.dt.float32r
AF = mybir.ActivationFunctionType
OP = mybir.AluOpType

B, S, D, M, C = 4, 4096, 768, 3072, 8921
SPARSITY_COEF = 1e-3
CP = 4480            # per-half padded label count (35 * 128)
CT = CP // 128       # 35 label tiles per core
Q = 4                # sequence quarters
SQ = S // Q          # 1024 tokens per quarter
NSB = SQ // 512      # 512-wide blocks per quarter
DT = D // 128        # 6 d-chunks
MT = M // 128        # 24 m-chunks
HALF0 = (C + 1) // 2  # 4461 labels on core h=0; C - HALF0 on h=1


def _build_nc():
    nc = bacc.Bacc(num_devices=8)
    xq = nc.dram_tensor("xq", [Q, 128, DT, SQ], F32, kind="ExternalInput")
    wenc = nc.dram_tensor("wenc", [MT, 128, DT, 128], F32, kind="ExternalInput")
    benc = nc.dram_tensor("benc", [128, MT], F32, kind="ExternalInput")
    wdec = nc.dram_tensor("wdec", [DT, 128, MT, 128], F32, kind="ExternalInput")
    bdec = nc.dram_tensor("bdec", [128, DT], F32, kind="ExternalInput")
    icdt = nc.dram_tensor("icdt", [CT, 128, MT, 128], F32, kind="ExternalInput")
    outwt = nc.dram_tensor("outwt", [CT, 128, DT, 128], F32, kind="ExternalInput")
    logits_o = nc.dram_tensor("logits_o", [128, CT], F32, kind="ExternalOutput")
    loss_o = nc.dram_tensor("loss_o", [1, 2], F32, kind="ExternalOutput")

    with tile.TileContext(nc) as tc:
        with tc.tile_pool(name="consts", bufs=1) as consts, \
             tc.tile_pool(name="xp", bufs=1) as xp, \
             tc.tile_pool(name="fp", bufs=1) as fp, \
             tc.tile_pool(name="wep", bufs=2) as wep, \
             tc.tile_pool(name="wdp", bufs=1) as wdp, \
             tc.tile_pool(name="icdp", bufs=2) as icdp, \
             tc.tile_pool(name="owp", bufs=2) as owp, \
             tc.tile_pool(name="ep", bufs=2) as ep, \
             tc.tile_pool(name="scp", bufs=2) as scp, \
             tc.tile_pool(name="stats", bufs=1) as stats, \
             tc.tile_pool(name="psum", bufs=2, space="PSUM") as psum, \
             tc.tile_pool(name="psl", bufs=1, space="PSUM") as psl:

            benc_t = consts.tile([128, MT], F32, tag="benc_t")
            nc.sync.dma_start(out=benc_t, in_=benc[:, :])
            bdec_t = consts.tile([128, DT], F32, tag="bdec_t")
            nc.sync.dma_start(out=bdec_t, in_=bdec[:, :])
            ones_t = consts.tile([128, 1], F32, tag="ones_t")
            nc.vector.memset(ones_t, 1.0)

            den_stage = stats.tile([128, CT * 8], F32, tag="den_stage")
            num_stage = stats.tile([128, CT * 8], F32, tag="num_stage")
            fsum_stage = stats.tile([128, MT * NSB * Q], F32, tag="fsum_stage")
            sq_stage = stats.tile([128, DT * NSB * Q], F32, tag="sq_stage")

            for q in range(Q):
                xq_t = xp.tile([128, DT, SQ], F32R, tag="xq_t")
                nc.sync.dma_start(out=xq_t, in_=xq[q, :, :, :].bitcast(F32R))
                f_t = fp.tile([128, MT, SQ], F32R, tag="f_t")

                # ---- encode: f = relu(w_enc.T @ x + b_enc), |f| accum ----
                for mt in range(MT):
                    we = wep.tile([128, DT, 128], F32R, tag="we")
                    nc.sync.dma_start(out=we, in_=wenc[mt, :, :, :].bitcast(F32R))
                    for sb in range(NSB):
                        pe_ = psum.tile([128, 512], F32, tag="pe_")
                        for dc in range(DT):
                            nc.tensor.matmul(pe_, lhsT=we[:, dc, :],
                                             rhs=xq_t[:, dc, sb * 512:(sb + 1) * 512],
                                             start=(dc == 0), stop=(dc == DT - 1))
                        nc.scalar.activation(
                            f_t[:, mt, sb * 512:(sb + 1) * 512], pe_, AF.Relu,
                            bias=benc_t[:, mt:mt + 1],
                            accum_out=fsum_stage[:, q * MT * NSB + mt * NSB + sb:
                                                 q * MT * NSB + mt * NSB + sb + 1])

                # ---- label attention over this quarter ----
                for ct in range(CT):
                    icd_t = icdp.tile([128, MT, 128], F32R, tag="icd_t")
                    nc.sync.dma_start(out=icd_t, in_=icdt[ct, :, :, :].bitcast(F32R))
                    ow_t = owp.tile([128, DT, 128], F32R, tag="ow_t")
                    nc.sync.dma_start(out=ow_t, in_=outwt[ct, :, :, :].bitcast(F32R))
                    for sb in range(NSB):
                        pa = psum.tile([128, 512], F32, tag="pa")
                        for k in range(MT):
                            nc.tensor.matmul(pa, lhsT=icd_t[:, k, :],
                                             rhs=f_t[:, k, sb * 512:(sb + 1) * 512],
                                             start=(k == 0), stop=(k == MT - 1))
                        py = psum.tile([128, 512], F32, tag="py")
                        for dc in range(DT):
                            nc.tensor.matmul(py, lhsT=ow_t[:, dc, :],
                                             rhs=xq_t[:, dc, sb * 512:(sb + 1) * 512],
                                             start=(dc == 0), stop=(dc == DT - 1))
                        col = ct * 8 + q * NSB + sb
                        e_t = ep.tile([128, 512], F32, tag="e_t")
                        nc.scalar.activation(e_t, pa, AF.Exp,
                                             accum_out=den_stage[:, col:col + 1])
                        sc = scp.tile([128, 512], F32, tag="sc")
                        nc.vector.scalar_tensor_tensor(
                            out=sc, in0=e_t, scalar=1.0, in1=py,
                            op0=OP.mult, op1=OP.mult,
                            accum_out=num_stage[:, col:col + 1])

                # ---- decode for SAE loss: (x_hat - x)^2 accum ----
                for dt in range(DT):
                    wd = wdp.tile([128, MT, 128], F32R, tag="wd")
                    nc.sync.dma_start(out=wd, in_=wdec[dt, :, :, :].bitcast(F32R))
                    for sb in range(NSB):
                        pd = psum.tile([128, 512], F32, tag="pe_")
                        for k in range(MT):
                            nc.tensor.matmul(pd, lhsT=wd[:, k, :],
                                             rhs=f_t[:, k, sb * 512:(sb + 1) * 512],
                                             start=(k == 0), stop=(k == MT - 1))
                        sc2 = scp.tile([128, 512], F32, tag="sc")
                        nc.vector.scalar_tensor_tensor(
                            out=sc2, in0=pd, scalar=bdec_t[:, dt:dt + 1],
                            in1=xq_t[:, dt, sb * 512:(sb + 1) * 512].bitcast(F32),
                            op0=OP.add, op1=OP.subtract)
                        sq_t = scp.tile([128, 512], F32, tag="sq_t")
                        nc.scalar.activation(
                            sq_t, sc2, AF.Square,
                            accum_out=sq_stage[:, q * DT * NSB + dt * NSB + sb:
                                               q * DT * NSB + dt * NSB + sb + 1])

            # ---- final: logits = num/den; loss partials via partition-sum ----
            den_red = stats.tile([128, CT], F32, tag="den_red")
            nc.vector.tensor_reduce(
                den_red, den_stage.rearrange("p (c s) -> p c s", s=8),
                axis=mybir.AxisListType.X, op=OP.add)
            num_red = stats.tile([128, CT], F32, tag="num_red")
            nc.vector.tensor_reduce(
                num_red, num_stage.rearrange("p (c s) -> p c s", s=8),
                axis=mybir.AxisListType.X, op=OP.add)
            rec = stats.tile([128, CT], F32, tag="rec")
            nc.vector.reciprocal(rec, den_red)
            lg = stats.tile([128, CT], F32, tag="lg")
            nc.vector.tensor_mul(lg, num_red, rec)
            nc.sync.dma_start(out=logits_o[:, :], in_=lg)

            pair = stats.tile([128, 2], F32, tag="pair")
            nc.vector.tensor_reduce(pair[:, 0:1], sq_stage,
                                    axis=mybir.AxisListType.X, op=OP.add)
            nc.vector.tensor_reduce(pair[:, 1:2], fsum_stage,
                                    axis=mybir.AxisListType.X, op=OP.add)
            pl = psl.tile([1, 2], F32, tag="pl")
            nc.tensor.matmul(pl, lhsT=ones_t, rhs=pair, start=True, stop=True)
            pls = stats.tile([1, 2], F32, tag="pls")
            nc.scalar.copy(pls, pl)
            nc.sync.dma_start(out=loss_o[:, :], in_=pls)

    nc.finalize()
    return nc


_NC_CACHE = None


def _get_nc():
    global _NC_CACHE
    if _NC_CACHE is None:
        _NC_CACHE = _build_nc()
    return _NC_CACHE


def _prep_core_inputs(x, w_enc, b_enc, w_dec, b_dec, icd_projection, out_w):
    """Host-side reshape/transpose into the layouts the kernel consumes."""
    wenc = np.ascontiguousarray(
        w_enc.reshape(DT, 128, MT, 128).transpose(2, 1, 0, 3))
    benc = np.ascontiguousarray(b_enc.reshape(MT, 128).T)
    wdec = np.ascontiguousarray(
        w_dec.reshape(MT, 128, DT, 128).transpose(2, 1, 0, 3))
    bdec = np.ascontiguousarray(b_dec.reshape(DT, 128).T)

    halves = []
    for h in range(2):
        rows = icd_projection[HALF0 * h: HALF0 * (h + 1) if h == 0 else C]
        icd_pad = np.zeros((CP, M), dtype=np.float32)
        icd_pad[:rows.shape[0]] = rows
        icdt = np.ascontiguousarray(
            icd_pad.reshape(CT, 128, MT, 128).transpose(0, 3, 2, 1))
        orows = out_w[HALF0 * h: HALF0 * (h + 1) if h == 0 else C]
        ow_pad = np.zeros((CP, D), dtype=np.float32)
        ow_pad[:orows.shape[0]] = orows
        outwt = np.ascontiguousarray(
            ow_pad.reshape(CT, 128, DT, 128).transpose(0, 3, 2, 1))
        halves.append((icdt, outwt))

    in_maps = []
    for core in range(8):
        b, h = core // 2, core % 2
        xb = np.ascontiguousarray(
            x[b].reshape(Q, SQ, DT, 128).transpose(0, 3, 2, 1))
        icdt, outwt = halves[h]
        in_maps.append({"xq": xb, "wenc": wenc, "benc": benc,
                        "wdec": wdec, "bdec": bdec,
                        "icdt": icdt, "outwt": outwt})
    return in_maps


def run(inputs, trace=False):
    nc = _get_nc()
    in_maps = _prep_core_inputs(
        inputs["x"], inputs["w_enc"], inputs["b_enc"], inputs["w_dec"],
        inputs["b_dec"], inputs["icd_projection"], inputs["out_w"])
    res = run_bass_kernel_spmd(nc, in_maps, list(range(8)), trace=trace)

    out_b = inputs["out_b"]
    logits = np.empty((B, C), dtype=np.float32)
    sq_total = 0.0
    f_total = 0.0
    for core in range(8):
        b, h = core // 2, core % 2
        arr = res.results[core]["logits_o"].T.reshape(CP)
        if h == 0:
            logits[b, :HALF0] = arr[:HALF0]
            sq, fs = res.results[core]["loss_o"][0]
            sq_total += float(sq)
            f_total += float(fs)
        else:
            logits[b, HALF0:] = arr[:C - HALF0]
    logits += out_b[None, :].astype(np.float32)
    loss = sq_total / (B * S * D) + SPARSITY_COEF * f_total / (B * S * M)
    return (logits, np.float32(loss)), res


def kernel(**inputs):
    (logits, loss), _ = run(inputs, trace=False)
    return logits, loss
